# revision 25
# baseline (speedup 1.0000x reference)
"""Trainium2 Bass kernel for nn_CausalSE: causal cumulative-average pooling
+ squeeze-excite gating, data-parallel over batch (one NeuronCore per batch
element).

Reference math per batch element (D=512, T=8192, chunk=16, Tc=512):
    avg    = cumsum(x, t) / (t+1)
    pooled = avg[:, 15::16]                          # [D, Tc]
    h      = relu(w1 @ pooled + b1)                  # [64, Tc]
    g      = sigmoid(w2 @ h + b2)                    # [D, Tc]
    out    = repeat(g, 16, t)[:, :T] * x

The kernel is HBM-bound: per core it streams x in and out once.  x
crosses HBM as fp16 (host converts both ways): ~16.8 MB at the ~400
GB/s per-core aggregate DMA rate => ~42us floor + ~10us of fixed NEFF
preamble/postamble, so every compute engine must stay under ~40us and
the per-block serial chain (load -> w1-matmul -> scan -> gate-matmul
-> sigmoid -> multiply -> store) must pipeline across blocks.

Structure (v1 was DVE-bound at ~55us busy; failed experiments: PSUM
same-address broadcast-accumulate output APs lose updates (RMW
hazard), strided moving-operand matmuls run ~5x slow):
  - Chunk sums ride the (otherwise idle) PE: w1 @ chunk_sum(x) ==
    chunk_sum(w1 @ x), so the PE computes Y = w1 @ x (4 ki
    accumulation steps into PSUM [64, 512] banks) and DVE
    windowed-reduces the 8x-smaller Y.
  - relu rides DVE as tensor_scalar(add b1, max 0); b2 is folded into
    the gate matmul (h gets a constant 1.0 row 64, w2 a b2 row), so
    the 4 per-block sigmoid+16x-upsample ACTIVATEs merge into ONE
    (saves the 352-cycle ACT fixed cost 18x and all ACT bias reads)
    and ACT only ever needs the sigmoid table set (a dummy 1-element
    sigmoid up front pulls the table load into the startup window).
  - Two-level software pipelining: the gate matmuls + sigmoid of
    block k are emitted after block k+1's Y-matmuls (PE runs the
    Y-stream dense, keeping the HAM clock-gate warm), and the gate
    multiplies + stores of block k are emitted two iterations behind
    (DVE never waits on ACT).
  - DMA: one load per t-block carrying all 4 d-tiles; block 0 is
    small (256 cols) and goes FIRST on the SP ring so compute ramps
    at ~10us; w1 is host-pre-swizzled partition-major (the naive
    (d p)->p d rearrange makes 128-byte descriptors that crawl); b1
    rides column 0 of the scale tensor.  Stores pair d-tiles: d01 on
    the SP ring, d23 on the ACT ring.  GpSimd issues nothing.
"""

import sys

for _p in ("/opt/trn_rl_repo",):
    if _p not in sys.path:
        sys.path.insert(0, _p)

import numpy as np

B, D, T = 8, 512, 8192
DH = 64          # bottleneck dim = D // 8
CS = 16          # chunksize
TC = T // CS     # 512 chunks
NCORES = 8
NDT = D // 128   # 4 partition tiles of x / out
SB = 512         # max Y-matmul sub-block (one PSUM bank of fp32)
TBLOCKS = [(0, 256), (256, 512), (768, 1024), (1792, 1536), (3328, 2048),
           (5376, 1792), (7168, 768), (7936, 256)]
TBMAX = 2048

_compiled_nc = None


def build_nc():
    import concourse.tile as tile
    from concourse import bacc, mybir

    f32 = mybir.dt.float32
    f16 = mybir.dt.float16
    AF = mybir.ActivationFunctionType
    ALU = mybir.AluOpType
    AX = mybir.AxisListType

    # Bacc (not plain Bass): its finalize() runs the TRN2 sync-wait
    # legalization (move_matmul_waits_to_ldweights / event-semaphore
    # splitting) that walrus codegen requires.
    nc = bacc.Bacc("TRN2", target_bir_lowering=False)
    x_d = nc.declare_dram_parameter("x", [D, T], f16, isOutput=False)
    w1p_d = nc.declare_dram_parameter("w1p", [128, NDT * DH], f16,
                                      isOutput=False)
    w2e_d = nc.declare_dram_parameter("w2e", [DH + 1, D], f16, isOutput=False)
    sclb_d = nc.declare_dram_parameter("sclb", [DH, TC + 1], f32,
                                       isOutput=False)
    out_d = nc.declare_dram_parameter("out", [D, T], f16, isOutput=True)

    with tile.TileContext(nc) as tc:
        with (
            tc.tile_pool(name="xres", bufs=1) as xres,
            tc.tile_pool(name="small", bufs=1) as small,
            tc.tile_pool(name="ups", bufs=3) as ups,
            tc.tile_pool(name="psum_y", bufs=1, space="PSUM") as psum_y,
            tc.tile_pool(name="psum_g", bufs=2, space="PSUM") as psum_g,
        ):
            # x resident in SBUF: [128, 4, 8192] fp16 = 8 MB
            xt = xres.tile([128, NDT, T], f16, tag="x", name="x")
            w1s = small.tile([128, NDT, DH], f16, tag="w1")
            w2s = small.tile([DH + 1, D], f16, tag="w2")
            sclb = small.tile([DH, TC + 1], f32, tag="sclb")
            b1s = sclb[:, 0:1]
            scl = sclb[:, 1:TC + 1]
            q = small.tile([DH, TC], f32, tag="q")      # per-chunk w1@x sums
            qs = small.tile([DH, TC], f32, tag="qs")    # causal prefix
            h32 = small.tile([DH, TC], f32, tag="h32")
            # h with a constant 1.0 row DH that turns the gate matmul's
            # extra w2-row (= b2) into the bias add
            h16 = small.tile([DH + 1, TC], f16, tag="h16")
            # one 4-bank Y tile: matmuls fill [DH, sb, :] slices (each
            # within one bank), ONE windowed reduce covers the whole block
            yp = psum_y.tile([DH, 4, SB], f32, tag="yp", name="yp")

            nc.vector.memset(h16[DH:DH + 1, :], 1.0)

            # Dummy 1-element sigmoid: forces the walrus-inserted
            # ACT_TABLE_LOAD for the sigmoid set to run during the startup
            # DMA window instead of stalling ACT before the first real
            # sigmoid mid-stream.
            dummy = small.tile([1, 2], f32, tag="dummy")
            nc.vector.memset(dummy[:], 0.0)
            nc.scalar.activation(dummy[:, 1:2], dummy[:, 0:1], AF.Sigmoid)

            def load_block(eng, t0, TB):
                eng.dma_start(
                    xt[:, :, t0:t0 + TB],
                    x_d[:, t0:t0 + TB].rearrange("(k p) t -> p k t", p=128),
                )

            # HBM is over-committed here: the compute pipeline wants loads
            # at ~230-300 GB/s AND stores at the same rate, but the per-NC
            # cap is ~360 total.  A greedy load schedule hogs the cap and
            # serializes all stores after it.  Instead only the first four
            # blocks' loads are issued up front (alternating rings so
            # completion order matches need order); the rest are emitted
            # inside the pipeline loop BETWEEN store issues, so each ring's
            # FIFO naturally alternates load/store at block cadence.
            load_block(nc.sync, *TBLOCKS[0])
            nc.sync.dma_start(
                w1s[:], w1p_d[:].rearrange("p (d h) -> p d h", d=NDT)
            )
            nc.sync.dma_start(sclb[:], sclb_d[:])
            nc.scalar.dma_start(w2s[:], w2e_d[:])
            load_block(nc.scalar, *TBLOCKS[1])
            load_block(nc.sync, *TBLOCKS[2])
            load_block(nc.scalar, *TBLOCKS[3])

            sbg = 0  # rotating PSUM bank assignment for Y sub-blocks

            def prefix_stage(tb):
                """Y-matmuls + merged reduce + scan + scale + bias-relu."""
                t0, TB = TBLOCKS[tb]
                CB = TB // CS
                c0 = t0 // CS
                nsb = -(-TB // SB)
                w = TB // nsb    # equal sub-widths, each within one bank
                for ki in range(NDT):
                    for sb in range(nsb):
                        ts = t0 + sb * w
                        nc.tensor.matmul(
                            yp[:, sb, :w],
                            w1s[:, ki, :],
                            xt[:, ki, ts:ts + w],
                            start=(ki == 0),
                            stop=(ki == NDT - 1),
                        )
                # chunk sums of Y: ONE windowed reduce straight off PSUM
                # covering all of this block's sub-banks
                nc.vector.reduce_sum(
                    q[:, c0:c0 + CB],
                    yp[:, :nsb, :w].rearrange("p s (c j) -> p s c j", j=CS),
                    axis=AX.X,
                )
                nc.vector.tensor_tensor_scan(
                    qs[:, c0:c0 + CB],
                    q[:, c0:c0 + CB],
                    q[:, c0:c0 + CB],
                    0.0 if tb == 0 else qs[:, c0 - 1:c0],
                    op0=ALU.add,
                    op1=ALU.bypass,
                )
                nc.vector.tensor_mul(
                    h32[:, c0:c0 + CB], qs[:, c0:c0 + CB], scl[:, c0:c0 + CB]
                )
                nc.vector.tensor_scalar(
                    h16[:DH, c0:c0 + CB], h32[:, c0:c0 + CB],
                    b1s, 0.0, op0=ALU.add, op1=ALU.max,
                )

            def gate_stage(tb):
                """4 gate matmuls (bias via the 1.0 h-row) + ONE merged
                sigmoid + 16x upsample ACTIVATE for all 4 d-tiles."""
                t0, TB = TBLOCKS[tb]
                CB = TB // CS
                c0 = t0 // CS
                gp = psum_g.tile([128, NDT, TBMAX // CS], f32, tag="g",
                                 name="gp")
                for di in range(NDT):
                    nc.tensor.matmul(
                        gp[:, di, :CB],
                        w2s[:, di * 128:(di + 1) * 128],
                        h16[:, c0:c0 + CB],
                        start=True,
                        stop=True,
                    )
                u = ups.tile([128, NDT, TBMAX], f16, tag="u", name="u")
                nc.scalar.activation(
                    u[:, :, :TB].rearrange("p k (c j) -> p k c j", j=CS),
                    gp[:, :, :CB].unsqueeze(3).broadcast_to(
                        [128, NDT, CB, CS]),
                    AF.Sigmoid,
                )
                return u

            def mult_half(tb, u, half, tail=False):
                """Gate multiplies + store for one d-tile pair (half 0 =
                d01 -> SP ring, half 1 = d23 -> ACT ring)."""
                t0, TB = TBLOCKS[tb]
                for di in (2 * half, 2 * half + 1):
                    xv = xt[:, di, t0:t0 + TB]
                    nc.vector.tensor_tensor(
                        xv, xv, u[:, di, :TB], op=ALU.mult
                    )
                deng = nc.sync if half == 0 else nc.scalar
                if tail:
                    for di in (2 * half, 2 * half + 1):
                        deng.dma_start(
                            out_d[di * 128:(di + 1) * 128, t0:t0 + TB],
                            xt[:, di, t0:t0 + TB],
                        )
                else:
                    deng.dma_start(
                        out_d[half * 256:(half + 1) * 256,
                              t0:t0 + TB].rearrange("(k p) t -> p k t",
                                                    p=128),
                        xt[:, 2 * half:2 * half + 2, t0:t0 + TB],
                    )

            NB = len(TBLOCKS)
            ulist = {}
            for tb in range(NB):
                # the ready multiplies go FIRST so the in-order DVE queue
                # never parks them (and their stores) behind this block's
                # load-gated reduce; d01 before the prefix, d23 after
                if tb >= 2:
                    mult_half(tb - 2, ulist[tb - 2], 0)
                    # late loads ride the rings BEHIND the store just
                    # issued -> load/store alternation at block cadence
                    if tb - 2 + 4 < NB:
                        load_block(nc.sync if tb % 2 == 0 else nc.scalar,
                                   *TBLOCKS[tb + 2])
                prefix_stage(tb)
                if tb >= 2:
                    mult_half(tb - 2, ulist.pop(tb - 2), 1)
                if tb >= 1:
                    ulist[tb - 1] = gate_stage(tb - 1)
            ulist[NB - 1] = gate_stage(NB - 1)
            mult_half(NB - 2, ulist[NB - 2], 0)
            mult_half(NB - 2, ulist.pop(NB - 2), 1)
            mult_half(NB - 1, ulist[NB - 1], 0, tail=True)
            mult_half(NB - 1, ulist.pop(NB - 1), 1, tail=True)
    # run_bass_via_pjrt serializes nc.m as-is; Bacc defers register
    # allocation and TRN2 sync-wait legalization to finalize(), so it must
    # run here or walrus rejects the BIR.
    nc.finalize()
    return nc


def _host_inputs(x, w1, b1, w2, b2, chunksize):
    x = np.asarray(x)
    w1 = np.asarray(w1, dtype=np.float32)
    b1 = np.ascontiguousarray(np.asarray(b1, dtype=np.float32))
    w2 = np.asarray(w2, dtype=np.float32)
    b2 = np.asarray(b2, dtype=np.float32)
    cs = int(chunksize)
    assert cs == CS and x.shape == (B, D, T), (cs, x.shape)
    x16 = np.ascontiguousarray(x.astype(np.float16))
    # w1 pre-swizzled partition-major: w1p[p, k*DH+h] = w1[h, k*128+p]
    w1p = np.ascontiguousarray(
        w1.T.astype(np.float16).reshape(NDT, 128, DH)
        .transpose(1, 0, 2).reshape(128, NDT * DH)
    )
    # w2 transposed with b2 as the extra row DH (paired with h's 1.0 row)
    w2e = np.ascontiguousarray(np.concatenate(
        [w2.T, b2[None, :]], axis=0).astype(np.float16))     # [DH+1, D]
    scale = 1.0 / (CS * np.arange(1, TC + 1, dtype=np.float32))
    sclb = np.ascontiguousarray(np.concatenate(
        [np.broadcast_to(b1[:, None], (DH, 1)),
         np.broadcast_to(scale, (DH, TC))], axis=1,
    ))
    shared = dict(w1p=w1p, w2e=w2e, sclb=sclb)
    return x16, shared


def kernel(x, w1, b1, w2, b2, chunksize):
    global _compiled_nc
    from concourse.bass_utils import run_bass_kernel_spmd

    x16, shared = _host_inputs(x, w1, b1, w2, b2, chunksize)
    if _compiled_nc is None:
        _compiled_nc = build_nc()
    in_maps = [
        {"x": np.ascontiguousarray(x16[i]), **shared} for i in range(NCORES)
    ]
    res = run_bass_kernel_spmd(_compiled_nc, in_maps, list(range(NCORES)))
    out = np.stack(
        [res.results[i]["out"] for i in range(NCORES)], axis=0
    ).astype(np.float32)
    return out


# revision 28
# speedup vs baseline: 1.0280x; 1.0280x over previous
"""Trainium2 Bass kernel for nn_CausalSE: causal cumulative-average pooling
+ squeeze-excite gating, data-parallel over batch (one NeuronCore per batch
element).

Reference math per batch element (D=512, T=8192, chunk=16, Tc=512):
    avg    = cumsum(x, t) / (t+1)
    pooled = avg[:, 15::16]                          # [D, Tc]
    h      = relu(w1 @ pooled + b1)                  # [64, Tc]
    g      = sigmoid(w2 @ h + b2)                    # [D, Tc]
    out    = repeat(g, 16, t)[:, :T] * x

The kernel is HBM-bound: per core it streams x in and out once.  x
crosses HBM as fp16 (host converts both ways): ~16.8 MB at the ~400
GB/s per-core aggregate DMA rate => ~42us floor + ~10us of fixed NEFF
preamble/postamble, so every compute engine must stay under ~40us and
the per-block serial chain (load -> w1-matmul -> scan -> gate-matmul
-> sigmoid -> multiply -> store) must pipeline across blocks.

Structure (v1 was DVE-bound at ~55us busy; failed experiments: PSUM
same-address broadcast-accumulate output APs lose updates (RMW
hazard), strided moving-operand matmuls run ~5x slow):
  - Chunk sums ride the (otherwise idle) PE: w1 @ chunk_sum(x) ==
    chunk_sum(w1 @ x), so the PE computes Y = w1 @ x (4 ki
    accumulation steps into PSUM [64, 512] banks) and DVE
    windowed-reduces the 8x-smaller Y.
  - relu rides DVE as tensor_scalar(add b1, max 0); b2 is folded into
    the gate matmul (h gets a constant 1.0 row 64, w2 a b2 row), so
    the 4 per-block sigmoid+16x-upsample ACTIVATEs merge into ONE
    (saves the 352-cycle ACT fixed cost 18x and all ACT bias reads)
    and ACT only ever needs the sigmoid table set (a dummy 1-element
    sigmoid up front pulls the table load into the startup window).
  - Two-level software pipelining: the gate matmuls + sigmoid of
    block k are emitted after block k+1's Y-matmuls (PE runs the
    Y-stream dense, keeping the HAM clock-gate warm), and the gate
    multiplies + stores of block k are emitted two iterations behind
    (DVE never waits on ACT).
  - DMA: one load per t-block carrying all 4 d-tiles; block 0 is
    small (256 cols) and goes FIRST on the SP ring so compute ramps
    at ~10us; w1 is host-pre-swizzled partition-major (the naive
    (d p)->p d rearrange makes 128-byte descriptors that crawl); b1
    rides column 0 of the scale tensor.  Stores pair d-tiles: d01 on
    the SP ring, d23 on the ACT ring.  GpSimd issues nothing.
"""

import sys

for _p in ("/opt/trn_rl_repo",):
    if _p not in sys.path:
        sys.path.insert(0, _p)

import numpy as np

B, D, T = 8, 512, 8192
DH = 64          # bottleneck dim = D // 8
CS = 16          # chunksize
TC = T // CS     # 512 chunks
NCORES = 8
NDT = D // 128   # 4 partition tiles of x / out
SB = 512         # max Y-matmul sub-block (one PSUM bank of fp32)
TBLOCKS = [(0, 256), (256, 512), (768, 1024), (1792, 1536), (3328, 2048),
           (5376, 1792), (7168, 768), (7936, 256)]
TBMAX = 2048

_compiled_nc = None


def build_nc():
    import concourse.tile as tile
    from concourse import bacc, mybir

    f32 = mybir.dt.float32
    f16 = mybir.dt.float16
    AF = mybir.ActivationFunctionType
    ALU = mybir.AluOpType
    AX = mybir.AxisListType

    # Bacc (not plain Bass): its finalize() runs the TRN2 sync-wait
    # legalization (move_matmul_waits_to_ldweights / event-semaphore
    # splitting) that walrus codegen requires.
    nc = bacc.Bacc("TRN2", target_bir_lowering=False)
    x_d = nc.declare_dram_parameter("x", [D, T], f16, isOutput=False)
    w1p_d = nc.declare_dram_parameter("w1p", [128, NDT * DH], f16,
                                      isOutput=False)
    w2e_d = nc.declare_dram_parameter("w2e", [DH + 1, D], f16, isOutput=False)
    sclb_d = nc.declare_dram_parameter("sclb", [DH, TC + 1], f32,
                                       isOutput=False)
    out_d = nc.declare_dram_parameter("out", [D, T], f16, isOutput=True)

    with tile.TileContext(nc) as tc:
        with (
            tc.tile_pool(name="xres", bufs=1) as xres,
            tc.tile_pool(name="small", bufs=1) as small,
            tc.tile_pool(name="ups", bufs=3) as ups,
            tc.tile_pool(name="psum_y", bufs=1, space="PSUM") as psum_y,
            tc.tile_pool(name="psum_g", bufs=2, space="PSUM") as psum_g,
        ):
            # x resident in SBUF: [128, 4, 8192] fp16 = 8 MB
            xt = xres.tile([128, NDT, T], f16, tag="x", name="x")
            w1s = small.tile([128, NDT, DH], f16, tag="w1")
            w2s = small.tile([DH + 1, D], f16, tag="w2")
            sclb = small.tile([DH, TC + 1], f32, tag="sclb")
            b1s = sclb[:, 0:1]
            scl = sclb[:, 1:TC + 1]
            q = small.tile([DH, TC], f32, tag="q")      # per-chunk w1@x sums
            qs = small.tile([DH, TC], f32, tag="qs")    # causal prefix
            h32 = small.tile([DH, TC], f32, tag="h32")
            # h with a constant 1.0 row DH that turns the gate matmul's
            # extra w2-row (= b2) into the bias add
            h16 = small.tile([DH + 1, TC], f16, tag="h16")
            # one 4-bank Y tile: matmuls fill [DH, sb, :] slices (each
            # within one bank), ONE windowed reduce covers the whole block
            yp = psum_y.tile([DH, 4, SB], f32, tag="yp", name="yp")

            nc.vector.memset(h16[DH:DH + 1, :], 1.0)

            def load_block(eng, t0, TB):
                eng.dma_start(
                    xt[:, :, t0:t0 + TB],
                    x_d[:, t0:t0 + TB].rearrange("(k p) t -> p k t", p=128),
                )

            # HBM is over-committed here: the compute pipeline wants loads
            # at ~230-300 GB/s AND stores at the same rate, but the per-NC
            # cap is ~360 total.  A greedy load schedule hogs the cap and
            # serializes all stores after it.  Instead only the first four
            # blocks' loads are issued up front (alternating rings so
            # completion order matches need order); the rest are emitted
            # inside the pipeline loop BETWEEN store issues, so each ring's
            # FIFO naturally alternates load/store at block cadence.
            load_block(nc.sync, *TBLOCKS[0])
            nc.sync.dma_start(
                w1s[:], w1p_d[:].rearrange("p (d h) -> p d h", d=NDT)
            )
            nc.sync.dma_start(sclb[:], sclb_d[:])
            nc.scalar.dma_start(w2s[:], w2e_d[:])
            load_block(nc.scalar, *TBLOCKS[1])
            load_block(nc.sync, *TBLOCKS[2])
            load_block(nc.scalar, *TBLOCKS[3])

            # Dummy 1-element sigmoid AFTER the startup DMA issues: pulls
            # the walrus-inserted sigmoid ACT_TABLE_LOAD into the load
            # window without delaying the ACT ring's early issues.
            dummy = small.tile([1, 2], f32, tag="dummy")
            nc.vector.memset(dummy[:], 0.0)
            nc.scalar.activation(dummy[:, 1:2], dummy[:, 0:1], AF.Sigmoid)

            sbg = 0  # rotating PSUM bank assignment for Y sub-blocks

            def prefix_stage(tb):
                """Y-matmuls + merged reduce + scan + scale + bias-relu."""
                t0, TB = TBLOCKS[tb]
                CB = TB // CS
                c0 = t0 // CS
                nsb = -(-TB // SB)
                w = TB // nsb    # equal sub-widths, each within one bank
                for ki in range(NDT):
                    for sb in range(nsb):
                        ts = t0 + sb * w
                        nc.tensor.matmul(
                            yp[:, sb, :w],
                            w1s[:, ki, :],
                            xt[:, ki, ts:ts + w],
                            start=(ki == 0),
                            stop=(ki == NDT - 1),
                        )
                # chunk sums of Y: ONE windowed reduce straight off PSUM
                # covering all of this block's sub-banks
                nc.vector.reduce_sum(
                    q[:, c0:c0 + CB],
                    yp[:, :nsb, :w].rearrange("p s (c j) -> p s c j", j=CS),
                    axis=AX.X,
                )
                nc.vector.tensor_tensor_scan(
                    qs[:, c0:c0 + CB],
                    q[:, c0:c0 + CB],
                    q[:, c0:c0 + CB],
                    0.0 if tb == 0 else qs[:, c0 - 1:c0],
                    op0=ALU.add,
                    op1=ALU.bypass,
                )
                nc.vector.tensor_mul(
                    h32[:, c0:c0 + CB], qs[:, c0:c0 + CB], scl[:, c0:c0 + CB]
                )
                nc.vector.tensor_scalar(
                    h16[:DH, c0:c0 + CB], h32[:, c0:c0 + CB],
                    b1s, 0.0, op0=ALU.add, op1=ALU.max,
                )

            def gate_stage(tb):
                """4 gate matmuls (bias via the 1.0 h-row) + ONE merged
                sigmoid + 16x upsample ACTIVATE for all 4 d-tiles."""
                t0, TB = TBLOCKS[tb]
                CB = TB // CS
                c0 = t0 // CS
                gp = psum_g.tile([128, NDT, TBMAX // CS], f32, tag="g",
                                 name="gp")
                for di in range(NDT):
                    nc.tensor.matmul(
                        gp[:, di, :CB],
                        w2s[:, di * 128:(di + 1) * 128],
                        h16[:, c0:c0 + CB],
                        start=True,
                        stop=True,
                    )
                u = ups.tile([128, NDT, TBMAX], f16, tag="u", name="u")
                nc.scalar.activation(
                    u[:, :, :TB].rearrange("p k (c j) -> p k c j", j=CS),
                    gp[:, :, :CB].unsqueeze(3).broadcast_to(
                        [128, NDT, CB, CS]),
                    AF.Sigmoid,
                )
                return u

            def mult_half(tb, u, half, tail=False):
                """Gate multiplies + store for one d-tile pair (half 0 =
                d01 -> SP ring, half 1 = d23 -> ACT ring)."""
                t0, TB = TBLOCKS[tb]
                for di in (2 * half, 2 * half + 1):
                    xv = xt[:, di, t0:t0 + TB]
                    nc.vector.tensor_tensor(
                        xv, xv, u[:, di, :TB], op=ALU.mult
                    )
                # d01 stores ride the SP ring (interleaved with the late
                # loads); d23 stores go out via gpsimd SWDGE -- a third,
                # fully independent DMA queue that never sits behind loads
                deng = nc.sync if half == 0 else nc.gpsimd
                if tail:
                    # tail: both HWDGE rings are idle, skip slow SWDGE
                    deng = nc.sync if half == 0 else nc.scalar
                    for di in (2 * half, 2 * half + 1):
                        deng.dma_start(
                            out_d[di * 128:(di + 1) * 128, t0:t0 + TB],
                            xt[:, di, t0:t0 + TB],
                        )
                else:
                    deng.dma_start(
                        out_d[half * 256:(half + 1) * 256,
                              t0:t0 + TB].rearrange("(k p) t -> p k t",
                                                    p=128),
                        xt[:, 2 * half:2 * half + 2, t0:t0 + TB],
                    )

            NB = len(TBLOCKS)
            ulist = {}
            for tb in range(NB):
                # the ready multiplies go FIRST so the in-order DVE queue
                # never parks them (and their stores) behind this block's
                # load-gated reduce; d01 before the prefix, d23 after
                if tb >= 2:
                    mult_half(tb - 2, ulist[tb - 2], 0)
                    # late loads ride the rings BEHIND the store just
                    # issued -> load/store alternation at block cadence
                    if tb - 2 + 4 < NB:
                        load_block(nc.sync if tb % 2 == 0 else nc.scalar,
                                   *TBLOCKS[tb + 2])
                prefix_stage(tb)
                if tb >= 2:
                    mult_half(tb - 2, ulist.pop(tb - 2), 1)
                if tb >= 1:
                    ulist[tb - 1] = gate_stage(tb - 1)
            ulist[NB - 1] = gate_stage(NB - 1)
            mult_half(NB - 2, ulist[NB - 2], 0)
            mult_half(NB - 2, ulist.pop(NB - 2), 1)
            mult_half(NB - 1, ulist[NB - 1], 0, tail=True)
            mult_half(NB - 1, ulist.pop(NB - 1), 1, tail=True)
    # run_bass_via_pjrt serializes nc.m as-is; Bacc defers register
    # allocation and TRN2 sync-wait legalization to finalize(), so it must
    # run here or walrus rejects the BIR.
    nc.finalize()
    return nc


def _host_inputs(x, w1, b1, w2, b2, chunksize):
    x = np.asarray(x)
    w1 = np.asarray(w1, dtype=np.float32)
    b1 = np.ascontiguousarray(np.asarray(b1, dtype=np.float32))
    w2 = np.asarray(w2, dtype=np.float32)
    b2 = np.asarray(b2, dtype=np.float32)
    cs = int(chunksize)
    assert cs == CS and x.shape == (B, D, T), (cs, x.shape)
    x16 = np.ascontiguousarray(x.astype(np.float16))
    # w1 pre-swizzled partition-major: w1p[p, k*DH+h] = w1[h, k*128+p]
    w1p = np.ascontiguousarray(
        w1.T.astype(np.float16).reshape(NDT, 128, DH)
        .transpose(1, 0, 2).reshape(128, NDT * DH)
    )
    # w2 transposed with b2 as the extra row DH (paired with h's 1.0 row)
    w2e = np.ascontiguousarray(np.concatenate(
        [w2.T, b2[None, :]], axis=0).astype(np.float16))     # [DH+1, D]
    scale = 1.0 / (CS * np.arange(1, TC + 1, dtype=np.float32))
    sclb = np.ascontiguousarray(np.concatenate(
        [np.broadcast_to(b1[:, None], (DH, 1)),
         np.broadcast_to(scale, (DH, TC))], axis=1,
    ))
    shared = dict(w1p=w1p, w2e=w2e, sclb=sclb)
    return x16, shared


def kernel(x, w1, b1, w2, b2, chunksize):
    global _compiled_nc
    from concourse.bass_utils import run_bass_kernel_spmd

    x16, shared = _host_inputs(x, w1, b1, w2, b2, chunksize)
    if _compiled_nc is None:
        _compiled_nc = build_nc()
    in_maps = [
        {"x": np.ascontiguousarray(x16[i]), **shared} for i in range(NCORES)
    ]
    res = run_bass_kernel_spmd(_compiled_nc, in_maps, list(range(NCORES)))
    out = np.stack(
        [res.results[i]["out"] for i in range(NCORES)], axis=0
    ).astype(np.float32)
    return out
